# revision 15
# baseline (speedup 1.0000x reference)
"""2-layer GATConv (PyG-style, edge_dim, self-loops fill='mean') on 8 TRN2 NeuronCores.

v2 strategy (vs baseline): the Q7 SWDGE descriptor generation (~8.5ns/desc)
was the bottleneck at ~7.7ms of 9.3ms; this version halves descriptor count
and trims stream padding:
  - NO per-edge a_dst gather: a_dst is expanded per edge on-device via a
    transposed one-hot (ind_T) built on DVE + small TensorE matmuls.
  - Exact per-(tile,chunk) gather sizes: num_idxs = cross-core max of the
    true edge count (tiles snake-dealt to cores in similar-size blocks of
    8), with partial last groups memset to zero instead of rounding the
    descriptor count up to 128.
  - Padding positions point at dedicated pad table rows whose a_src is
    patched to -1000 in phase 0, so exp(leaky(alpha_pad)) underflows and
    pads self-suppress without correction terms.
Everything else (phase-0 fused xh|a_src|a_dst matmul, AllGather of the xh
table, one-hot segment-sum matmuls, analytic self-loop) as the baseline.
"""

import os
import sys

sys.path.insert(0, "/opt/trn_rl_repo")

import numpy as np
import ml_dtypes

import concourse.bass as bass
import concourse.mybir as mybir
from concourse import bacc, tile
from concourse.bass_utils import run_bass_kernel_spmd
from concourse.masks import make_identity

F32 = mybir.dt.float32
BF16 = mybir.dt.bfloat16
I16 = mybir.dt.int16
BF = ml_dtypes.bfloat16

NCORES = 8
H = 4
CH = 32          # channels per head
F = 128          # hidden/out features
ED = 16          # edge feature dim
TROW = 132       # table row elems used: 128 xh + 4 a_src
TSTRIDE = 256    # table row stride in elems (512B; gather stride must be %256B)
MROW = F + 2 * H
SENT = 512.0     # dst-slot sentinel for padded/garbage stream positions
WSPAN = 32768    # int16 gather window span (rows)
PAD_ASRC = -1000.0


def _cdiv(a, b):
    return -(-a // b)


def _wrap16(arr):
    """[L] (L%16==0) -> wrapped idx layout [128, L//16] (replicated x8)."""
    L = arr.shape[0]
    w = arr.reshape(L // 16, 16).T
    return np.ascontiguousarray(np.tile(w, (8, 1)))


# --------------------------------------------------------------------------
# host-side index preprocessing
# --------------------------------------------------------------------------

def _preprocess(x, src, dst, edge_attr):
    N = x.shape[0]
    E = src.shape[0]
    TPC = _cdiv(N, NCORES * 128)
    NT = NCORES * TPC
    NPC = TPC * 128
    NTOT = NT * 128
    NCHK = _cdiv(NTOT, WSPAN)

    deg = np.bincount(dst, minlength=N)
    order = np.argsort(-deg, kind="stable")
    tile_sorted = np.full(N, -1, np.int64)
    tile_sorted[order] = np.arange(N) // 128

    # tiles -> (core, index): rank by edge count, block of 8 per index (snake)
    cnt_st = np.bincount(tile_sorted[dst], minlength=NT)
    ranks = np.argsort(-cnt_st, kind="stable")
    core_of_st = np.zeros(NT, np.int64)
    tidx_of_st = np.zeros(NT, np.int64)
    for i in range(TPC):
        blk = ranks[i * NCORES:(i + 1) * NCORES]
        cs = range(NCORES) if i % 2 == 0 else range(NCORES - 1, -1, -1)
        for c, st in zip(cs, blk):
            core_of_st[st] = c
            tidx_of_st[st] = i

    nst = np.maximum(
        np.minimum(np.arange(NT) * 128 + 128, N) - np.arange(NT) * 128, 0)
    perm = np.full((NCORES, TPC, 128), -1, np.int64)
    for st in range(NT):
        nodes = order[st * 128: st * 128 + nst[st]]
        perm[core_of_st[st], tidx_of_st[st], :nst[st]] = nodes

    # pad slots (uniform across cores): (c, 0, 127) and (c, TPC-1, 127)
    patch = sorted({(0, 127), (TPC - 1, 127)})
    pad_pos = [(c, t, s) for c in range(NCORES) for (t, s) in patch]
    spares = [tuple(s) for s in np.argwhere(perm < 0)]
    spares = [s for s in spares if s not in set(pad_pos)]
    si = 0
    for (c, t, s) in pad_pos:
        v = perm[c, t, s]
        if v >= 0:
            perm[spares[si]] = v
            si += 1
        perm[c, t, s] = -1
    pad_gids = np.array(sorted(c * NPC + t * 128 + s for (c, t, s) in pad_pos),
                        np.int64)

    gid = np.full(N, -1, np.int64)
    flat = perm.reshape(-1)
    m = flat >= 0
    gid[flat[m]] = np.arange(NTOT)[m]
    assert (gid >= 0).all()

    d_gid = gid[dst]
    tile_e = d_gid // 128            # global tile g = core*TPC + tidx
    slot_e = d_gid % 128
    s_gid = gid[src]
    chunk_e = s_gid // WSPAN

    # uniform per-index per-chunk sizes: cross-core max of true counts
    cntc = np.bincount(tile_e * NCHK + chunk_e,
                       minlength=NT * NCHK).reshape(NCORES, TPC, NCHK)
    n_ic = cntc.max(axis=0)                       # [TPC, NCHK]
    G_ic = _cdiv(n_ic, 128)
    G_i = G_ic.sum(axis=1)                        # groups per tile index
    G_i = np.maximum(G_i, 1)
    Q_i = _cdiv(G_i, 8)
    off_ic = np.zeros((TPC, NCHK + 1), np.int64)
    off_ic[:, 1:] = np.cumsum(G_ic, axis=1)
    s16_ic = _cdiv(_cdiv(n_ic, 16), 16) * 16      # idx cols per section (32B-aligned)

    # sections meta per index: (ch, n, G, goff, base, span)
    sections = []
    for i in range(TPC):
        sec = []
        for ch in range(NCHK):
            if n_ic[i, ch] == 0:
                continue
            base = ch * WSPAN
            sec.append((ch, int(n_ic[i, ch]), int(G_ic[i, ch]),
                        int(off_ic[i, ch]), base, min(WSPAN, NTOT - base)))
        sections.append(sec)

    # pad row (window-local idx) per chunk
    pad_loc = np.zeros(NCHK, np.int64)
    for ch in range(NCHK):
        cand = pad_gids[(pad_gids >= ch * WSPAN) & (pad_gids < (ch + 1) * WSPAN)]
        assert len(cand) > 0
        pad_loc[ch] = cand[0] - ch * WSPAN

    # edge order: by (tile, chunk implicit via gid, gid)
    korder = np.lexsort((s_gid, tile_e))
    te_s = tile_e[korder]
    sg_s = s_gid[korder]
    tstart = np.searchsorted(te_s, np.arange(NT + 1))

    auxcats, dltcats, xts = [], [], []
    for c in range(NCORES):
        pieces, dpieces = [], []
        for i in range(TPC):
            g = c * TPC + i
            lo, hi = int(tstart[g]), int(tstart[g + 1])
            run_e = korder[lo:hi]
            run_g = sg_s[lo:hi]
            S = 128 * int(G_i[i])
            dl = np.full(S, SENT, np.float32)
            ets = np.full(S, -1, np.int64)
            iparts = []
            pos = 0
            for (ch, n, Gch, goff, base, span) in sections[i]:
                hi_c = int(np.searchsorted(run_g, (ch + 1) * WSPAN, side="left"))
                cnt = hi_c - pos
                assert 0 <= cnt <= n
                e_sec = run_e[pos:pos + cnt]
                pos = hi_c
                iv = np.full(s16_ic[i, ch] * 16, -1, np.int64)
                iv[:cnt] = run_g[hi_c - cnt:hi_c] - base
                iv[cnt:n] = pad_loc[ch]
                iparts.append(iv.astype(np.int16))
                j0 = goff * 128
                dl[j0:j0 + cnt] = slot_e[e_sec]
                ets[j0:j0 + cnt] = e_sec
            assert pos == hi - lo
            iw = (_wrap16(np.concatenate(iparts)) if iparts
                  else np.zeros((128, 0), np.int16))
            dl3 = np.ascontiguousarray(
                dl.reshape(int(G_i[i]), 128).T).astype(BF)
            vm = ets >= 0
            ea = np.where(vm[:, None], edge_attr[np.where(vm, ets, 0)], 0.0)
            Q = int(Q_i[i])
            eap = np.zeros((Q * 8 * 128, ED), np.float32)
            eap[:S] = ea
            eaT = (eap.reshape(Q, 8, 128, ED).transpose(1, 3, 0, 2)
                   .reshape(128, Q * 128)).astype(BF)
            pieces.append(np.concatenate(
                [iw.view(np.uint16), dl3.view(np.uint16), eaT.view(np.uint16)],
                axis=1))
            dpieces.append(dl.astype(BF).view(np.uint16)[None, :])
        auxcats.append(np.ascontiguousarray(np.concatenate(pieces, axis=1)))
        dltcats.append(np.ascontiguousarray(np.concatenate(dpieces, axis=1)))

        pc = perm[c].reshape(-1)
        xp = np.zeros((NPC, F), np.float32)
        mk = pc >= 0
        xp[mk] = x[pc[mk]]
        xts.append(np.ascontiguousarray(xp.T))

    deg_slot = np.where(perm >= 0, deg[np.where(perm >= 0, perm, 0)], 0)
    cntinv = (1.0 / np.maximum(deg_slot, 1)).astype(np.float32)
    cntinv_t = np.ascontiguousarray(cntinv.transpose(0, 2, 1))  # [C,128,TPC]

    AW = auxcats[0].shape[1]
    DW = dltcats[0].shape[1]
    assert all(a.shape[1] == AW for a in auxcats)
    assert all(d.shape[1] == DW for d in dltcats)

    return dict(N=N, E=E, TPC=TPC, NT=NT, NPC=NPC, NTOT=NTOT, NCHK=NCHK,
                G_i=G_i, Q_i=Q_i, s16_i=s16_ic.sum(axis=1),
                sections=sections, s16_ic=s16_ic, AW=AW, DW=DW,
                perm=perm, auxcats=auxcats, dltcats=dltcats, xts=xts,
                cntinv=cntinv_t, patch=patch)


def _blockdiag(att):
    out = np.zeros((F, H), dtype=np.float32)
    for h in range(H):
        out[h * CH:(h + 1) * CH, h] = att[h]
    return out


def _raw_dma_gather(gp, out_ap, in_ap, idxs_ap, num_idxs, elem_size,
                    elem_step=None, queue_num=0):
    from concourse import ap_utils
    from concourse._compat import exact_div
    assert idxs_ap.dtype == mybir.dt.int16
    assert in_ap.dtype == out_ap.dtype
    if elem_step is None:
        assert ap_utils.ap_is_contiguous(in_ap.ap[1:])
        elem_step = elem_size
    assert ap_utils.ap_is_contiguous(out_ap.ap[1:])
    assert ap_utils.ap_is_contiguous(idxs_ap.ap[1:])
    assert in_ap.ap[-1][1] == out_ap.ap[-1][1] == elem_size
    assert in_ap.ap[0][0] == elem_step
    stride_bytes = elem_step * mybir.dt.size(in_ap.dtype)
    stride_bytes_256 = exact_div(stride_bytes, 256)
    _in_ap = gp.lower_ap_dma(in_ap, for_custom_bir_dma=True)
    _idxs_ap = gp.lower_ap(idxs_ap)
    _out_ap = gp.lower_ap(out_ap)
    return gp.add_instruction(
        mybir.InstDMAGatherAnt(
            name=gp.bass.get_next_instruction_name(),
            ins=[*_in_ap, _idxs_ap,
                 gp.lower_val_access(gp.to_reg(num_idxs))],
            outs=[_out_ap],
            transpose=False, num_idxs=num_idxs, elem_size=elem_size,
            stride_bytes_256=stride_bytes_256, gen_mode=0,
            single_packet=False, queue_num=queue_num,
            sbuf_tokens_per_rank=0, sbuf_free_dim_per_rank=0,
            sbuf_free_dim_pad_per_rank=0, sbuf_byte_offset=0,
        ))


# --------------------------------------------------------------------------
# device program (single SPMD program: shapes uniform across cores)
# --------------------------------------------------------------------------

def _build(meta):
    TPC, NPC, NTOT, AW, DW = (meta[k] for k in ("TPC", "NPC", "NTOT", "AW", "DW"))
    G_i, Q_i, s16_i = meta["G_i"], meta["Q_i"], meta["s16_i"]
    sections = meta["sections"]
    s16_ic = meta["s16_ic"]
    patch = meta["patch"]
    GMAX = int(G_i.max())
    QMAX = int(Q_i.max())
    SMAX16 = int(s16_i.max())

    nc = bacc.Bacc("TRN2", target_bir_lowering=False, debug=False,
                   num_devices=NCORES)

    def din(name, shape, dt):
        return nc.dram_tensor(name, list(shape), dt, kind="ExternalInput")

    xT_d = din("xT", (F, NPC), F32)
    aux_d = din("aux", (128, AW), I16)
    dlt_d = din("dlt", (1, DW), I16)
    cntinv_d = din("cntinv", (128, TPC), F32)
    padc_d = din("padc", (1, H), BF16)
    Wp = [din(f"W{l}", (F, F), F32) for l in (1, 2)]
    WTp = [din(f"WT{l}", (F, F), F32) for l in (1, 2)]
    Asdp = [din(f"Asd{l}", (F, 2 * H), F32) for l in (1, 2)]
    Aep = [din(f"Ae{l}", (F, H), F32) for l in (1, 2)]
    WeTp = [din(f"WeT{l}", (F, ED), F32) for l in (1, 2)]
    biasp = [din(f"b{l}", (1, F), F32) for l in (1, 2)]
    out_d = nc.dram_tensor("out", [NPC, F], F32, kind="ExternalOutput")

    ltab_d = nc.dram_tensor("ltab", [NPC, TSTRIDE], BF16)
    gtab_d = nc.dram_tensor("gtab", [NTOT, TSTRIDE], BF16, addr_space="Shared")
    ltab2_d = nc.dram_tensor("ltab2", [NPC, TSTRIDE], BF16)
    gtab2_d = nc.dram_tensor("gtab2", [NTOT, TSTRIDE], BF16, addr_space="Shared")
    hT_d = nc.dram_tensor("hT", [F, NPC], F32)

    rg = [list(range(NCORES))]

    with tile.TileContext(nc) as tc:
        with (
            tc.tile_pool(name="persist", bufs=1) as pp,
            tc.tile_pool(name="sb", bufs=2) as sb,
            tc.tile_pool(name="sbg", bufs=3) as sbg,
            tc.tile_pool(name="ps", bufs=2, space="PSUM") as ps,     # ph0/acc/trp
            tc.tile_pool(name="ps1", bufs=2, space="PSUM") as ps1,   # pae+adstE
            tc.tile_pool(name="ps2", bufs=1, space="PSUM") as ps2,   # dl replicate
        ):
            ident = pp.tile([128, 128], F32)
            make_identity(nc, ident[:])
            iota_i = pp.tile([128, 128], mybir.dt.int32, tag="ioti")
            nc.gpsimd.iota(iota_i[:], pattern=[[1, 128]], base=0,
                           channel_multiplier=0)
            iotaRow = pp.tile([128, 128], BF16)
            nc.vector.tensor_copy(iotaRow[:], iota_i[:])
            iota_pi = pp.tile([128, 1], mybir.dt.int32, tag="iopi")
            nc.gpsimd.iota(iota_pi[:], pattern=[[1, 1]], base=0,
                           channel_multiplier=1)
            iota_p = pp.tile([128, 1], F32, tag="iop")
            nc.vector.tensor_copy(iota_p[:], iota_pi[:])

            loctab = pp.tile([128, TPC, TROW], BF16, tag="loctab")
            asd_all = pp.tile([128, TPC, 2 * H], F32)
            ones_sb = pp.tile([1, 128], F32, tag="ones")
            nc.vector.memset(ones_sb[:], 1.0)
            ones_bf = pp.tile([1, 128], BF16, tag="onesbf")
            nc.vector.memset(ones_bf[:], 1.0)
            bias_full = pp.tile([128, F], F32, tag="biasf")
            bias_sb = pp.tile([1, F], F32, tag="bias")
            cinv_sb = pp.tile([128, TPC], F32, tag="cinv")
            nc.sync.dma_start(out=cinv_sb[:], in_=cntinv_d[:, :])
            for _r in range(3):
                xgz = sbg.tile([128, GMAX, TROW], BF16, tag="xg")
                nc.vector.memset(xgz[:], 0)

            for li in range(2):
                layer1 = li == 0
                ltab = ltab_d if layer1 else ltab2_d
                gtab = gtab_d if layer1 else gtab2_d

                # ---- layer weight prep ----
                wet_sb = sb.tile([F, ED], F32, tag="wet_sb")
                nc.sync.dma_start(out=wet_sb[:], in_=WeTp[li][:, :])
                ae_sb = sb.tile([F, H], F32, tag="ae_sb")
                nc.sync.dma_start(out=ae_sb[:], in_=Aep[li][:, :])
                wae_ps = ps.tile([ED, H], F32, tag="ph0")
                nc.tensor.matmul(out=wae_ps[:], lhsT=wet_sb[:],
                                 rhs=ae_sb[:], start=True, stop=True)
                wae_sb = sb.tile([ED, H], BF16, tag="wae_sb")
                nc.vector.tensor_copy(wae_sb[:], wae_ps[:])
                wae_rep = sb.tile([128, 32], BF16, tag="wae_rep")
                nc.vector.memset(wae_rep[:], 0)
                for b in range(8):
                    nc.gpsimd.dma_start(
                        out=wae_rep[16 * b:16 * b + 16, 4 * b:4 * b + 4],
                        in_=wae_sb[:])

                wt_sb = sb.tile([F, F], F32, tag="wt_sb")
                nc.sync.dma_start(out=wt_sb[:], in_=WTp[li][:, :])
                asd_sb = sb.tile([F, 2 * H], F32, tag="asd_sb")
                nc.sync.dma_start(out=asd_sb[:], in_=Asdp[li][:, :])
                wasd_ps = ps.tile([F, 2 * H], F32, tag="ph0")
                nc.tensor.matmul(out=wasd_ps[:], lhsT=wt_sb[:],
                                 rhs=asd_sb[:], start=True, stop=True)
                wcomb = sb.tile([F, F + 2 * H], F32, tag="wcomb")
                nc.sync.dma_start(out=wcomb[:, 0:F], in_=Wp[li][:, :])
                nc.vector.tensor_copy(wcomb[:, F:F + 2 * H], wasd_ps[:])

                nc.sync.dma_start(out=bias_sb[:], in_=biasp[li][:, :])
                bias_ps = ps.tile([128, F], F32, tag="ph0")
                nc.tensor.matmul(out=bias_ps[:], lhsT=ones_sb[:],
                                 rhs=bias_sb[:], start=True, stop=True)
                nc.vector.tensor_copy(bias_full[:], bias_ps[:])

                # ---- phase 0 ----
                for t in range(TPC):
                    xt = sb.tile([128, 128], F32, tag="xt")
                    src_slab = xT_d if layer1 else hT_d
                    nc.sync.dma_start(out=xt[:],
                                      in_=src_slab[:, t * 128:(t + 1) * 128])
                    ph0 = ps.tile([128, F + 2 * H], F32, tag="ph0")
                    nc.tensor.matmul(out=ph0[:], lhsT=xt[:], rhs=wcomb[:],
                                     start=True, stop=True)
                    nc.scalar.activation(
                        out=loctab[:, t, :], in_=ph0[:, 0:TROW],
                        func=mybir.ActivationFunctionType.Copy)
                    for (pt, psl) in patch:
                        if pt == t:
                            nc.sync.dma_start(
                                out=loctab[psl:psl + 1, t, F:F + H],
                                in_=padc_d[0:1, :])
                    nc.vector.tensor_copy(asd_all[:, t, :],
                                          ph0[:, F:F + 2 * H])
                    nc.sync.dma_start(out=ltab[t * 128:(t + 1) * 128, 0:TROW],
                                      in_=loctab[:, t, :])

                # ---- all-gather the xh table ----
                nc.gpsimd.collective_compute(
                    "AllGather", mybir.AluOpType.bypass, replica_groups=rg,
                    ins=[ltab[:, :].opt()], outs=[gtab[:, :].opt()])

                # ---- phase 1/2 per dst tile ----
                aoff = 0
                doff = 0
                for t in range(TPC):
                    G = int(G_i[t])
                    Q = int(Q_i[t])
                    S = 128 * G
                    S16 = int(s16_i[t])
                    AWT = S16 + G + 128 * Q

                    aux_t = sbg.tile([128, SMAX16 + GMAX + 128 * QMAX], I16,
                                     tag="aux")
                    nc.scalar.dma_start(out=aux_t[:, 0:AWT],
                                        in_=aux_d[:, aoff:aoff + AWT])
                    dl_ap = aux_t[:, S16:S16 + G].bitcast(BF16)
                    eap_ap = aux_t[:, S16 + G:AWT].bitcast(BF16)
                    dlt_t = sb.tile([1, 128 * GMAX], BF16, tag="dlt")
                    nc.scalar.dma_start(
                        out=dlt_t[0:1, 0:S],
                        in_=dlt_d[0:1, doff:doff + S].bitcast(BF16))
                    aoff += AWT
                    doff += S

                    xg = sbg.tile([128, GMAX, TROW], BF16, tag="xg")
                    i16off = 0
                    for (ch, n, Gch, goff, base, span) in sections[t]:
                        _raw_dma_gather(
                            nc.gpsimd,
                            xg[:, goff:goff + Gch, :],
                            gtab[base:base + span, 0:TROW],
                            aux_t[:, i16off:i16off + _cdiv(n, 16)],
                            n, TROW, elem_step=TSTRIDE)
                        i16off += _cdiv(_cdiv(n, 16), 16) * 16
                    assert i16off == S16

                    # ind one-hot [p, g, s] for the segment-sum
                    ind = sbg.tile([128, GMAX, 128], BF16, tag="ind")
                    nc.vector.tensor_tensor(
                        out=ind[:, 0:G, :],
                        in0=dl_ap.unsqueeze(2).to_broadcast([128, G, 128]),
                        in1=iotaRow[:].unsqueeze(1).to_broadcast([128, G, 128]),
                        op=mybir.AluOpType.is_equal)

                    # ind_T one-hot [s, (g,p)] for the a_dst expansion
                    indT = sbg.tile([128, GMAX * 128], BF16, tag="indT")
                    for j0 in range(0, S, 512):
                        w = min(512, S - j0)
                        rep = ps2.tile([128, 512], F32, tag="rep")
                        nc.tensor.matmul(out=rep[:, 0:w], lhsT=ones_bf[:],
                                         rhs=dlt_t[0:1, j0:j0 + w],
                                         start=True, stop=True)
                        nc.vector.tensor_scalar(
                            out=indT[:, j0:j0 + w], in0=rep[:, 0:w],
                            scalar1=iota_p[:], scalar2=None,
                            op0=mybir.AluOpType.is_equal)

                    # pae+adstE share one PSUM bank tile
                    peA = ps1.tile([128, QMAX * 32 + GMAX * H], F32, tag="pea")
                    # a_dst per edge
                    adst_bf = sb.tile([128, H], BF16, tag="adstbf")
                    nc.scalar.activation(
                        out=adst_bf[:], in_=asd_all[:, t, H:2 * H],
                        func=mybir.ActivationFunctionType.Copy)
                    adstE = peA[:, QMAX * 32:].rearrange(
                        "p (g h) -> p g h", h=H)
                    for k in range(G):
                        nc.tensor.matmul(out=adstE[:, k, :],
                                         lhsT=indT[:, k * 128:(k + 1) * 128],
                                         rhs=adst_bf[:], start=True, stop=True)

                    # a_edge via packed blockdiag matmul
                    pae = peA[:, 0:QMAX * 32].rearrange(
                        "p (q b) -> p q b", b=32)
                    for q in range(Q):
                        nc.tensor.matmul(out=pae[:, q, :],
                                         lhsT=eap_ap[:, q * 128:(q + 1) * 128],
                                         rhs=wae_rep[:], start=True, stop=True)
                    pav = peA[:, 0:QMAX * 32].rearrange(
                        "p (qb h) -> p qb h", h=H)

                    # alpha -> p
                    z = sb.tile([128, GMAX, H], F32, tag="z")
                    nc.vector.tensor_add(z[:, 0:G, :], pav[:, 0:G, :],
                                         xg[:, 0:G, F:TROW])
                    nc.vector.tensor_add(z[:, 0:G, :], z[:, 0:G, :],
                                         adstE[:, 0:G, :])
                    zl = sb.tile([128, GMAX, H], F32, tag="zl")
                    nc.scalar.activation(
                        out=zl[:, 0:G, :], in_=z[:, 0:G, :],
                        func=mybir.ActivationFunctionType.Copy, scale=0.2)
                    nc.vector.tensor_max(z[:, 0:G, :], z[:, 0:G, :],
                                         zl[:, 0:G, :])
                    p_t = sb.tile([128, GMAX, H], F32, tag="p")
                    nc.scalar.activation(out=p_t[:, 0:G, :], in_=z[:, 0:G, :],
                                         func=mybir.ActivationFunctionType.Exp)

                    # msgstat = [p*xh | p | a_edge]
                    msgstat = sbg.tile([128, GMAX, MROW], BF16, tag="msgstat")
                    nc.vector.tensor_tensor(
                        out=msgstat[:, 0:G, 0:F].rearrange(
                            "p g (h c) -> p g h c", c=CH),
                        in0=xg[:, 0:G, 0:F].rearrange(
                            "p g (h c) -> p g h c", c=CH),
                        in1=p_t[:, 0:G, :].unsqueeze(3).to_broadcast(
                            [128, G, H, CH]),
                        op=mybir.AluOpType.mult)
                    nc.scalar.activation(
                        out=msgstat[:, 0:G, F:F + H], in_=p_t[:, 0:G, :],
                        func=mybir.ActivationFunctionType.Copy)
                    nc.scalar.activation(
                        out=msgstat[:, 0:G, F + H:MROW], in_=pav[:, 0:G, :],
                        func=mybir.ActivationFunctionType.Copy)

                    acc = ps.tile([128, MROW], F32, tag="acc")
                    for k in range(G):
                        nc.tensor.matmul(out=acc[:], lhsT=ind[:, k, :],
                                         rhs=msgstat[:, k, :],
                                         start=(k == 0), stop=(k == G - 1))

                    # ---- phase 2: self loop + normalize ----
                    sl = sb.tile([128, 3 * H], F32, tag="sl")
                    nc.vector.tensor_scalar_mul(sl[:, 0:H],
                                                acc[:, F + H:MROW],
                                                cinv_sb[:, t:t + 1])
                    nc.vector.tensor_add(sl[:, 0:H], sl[:, 0:H],
                                         asd_all[:, t, 0:H])
                    nc.vector.tensor_add(sl[:, 0:H], sl[:, 0:H],
                                         asd_all[:, t, H:2 * H])
                    nc.vector.tensor_scalar_mul(sl[:, H:2 * H], sl[:, 0:H], 0.2)
                    nc.vector.tensor_max(sl[:, 0:H], sl[:, 0:H], sl[:, H:2 * H])
                    nc.scalar.activation(out=sl[:, H:2 * H], in_=sl[:, 0:H],
                                         func=mybir.ActivationFunctionType.Exp)
                    nc.vector.tensor_add(sl[:, 2 * H:3 * H], acc[:, F:F + H],
                                         sl[:, H:2 * H])
                    nc.vector.tensor_scalar_add(sl[:, 2 * H:3 * H],
                                                sl[:, 2 * H:3 * H], 1e-16)
                    nc.vector.reciprocal(sl[:, 2 * H:3 * H], sl[:, 2 * H:3 * H])

                    of = sb.tile([128, F], F32, tag="of")
                    of4 = of[:].rearrange("p (h c) -> p h c", c=CH)
                    nc.vector.tensor_tensor(
                        out=of4,
                        in0=loctab[:, t, 0:F].rearrange(
                            "p (h c) -> p h c", c=CH),
                        in1=sl[:, H:2 * H].unsqueeze(2).to_broadcast(
                            [128, H, CH]),
                        op=mybir.AluOpType.mult)
                    nc.vector.tensor_add(of[:], of[:], acc[:, 0:F])
                    nc.vector.tensor_tensor(
                        out=of4, in0=of4,
                        in1=sl[:, 2 * H:3 * H].unsqueeze(2).to_broadcast(
                            [128, H, CH]),
                        op=mybir.AluOpType.mult)
                    nc.vector.tensor_add(of[:], of[:], bias_full[:])

                    if layer1:
                        nc.vector.tensor_scalar_max(of[:], of[:], 0.0)
                        trp = ps.tile([128, F + 2 * H], F32, tag="ph0")
                        nc.tensor.transpose(out=trp[:, 0:128], in_=of[:],
                                            identity=ident[:])
                        trs = sb.tile([128, 128], F32, tag="trs")
                        nc.vector.tensor_copy(trs[:], trp[:, 0:128])
                        nc.sync.dma_start(out=hT_d[:, t * 128:(t + 1) * 128],
                                          in_=trs[:])
                    else:
                        nc.sync.dma_start(out=out_d[t * 128:(t + 1) * 128, :],
                                          in_=of[:])

    nc.compile()
    return nc


# --------------------------------------------------------------------------
# entry point
# --------------------------------------------------------------------------

def _make_in_maps(meta, inputs):
    wmaps = {}
    for li in (1, 2):
        W = np.asarray(inputs[f"W{li}"], np.float32)
        wmaps[f"W{li}"] = W
        wmaps[f"WT{li}"] = np.ascontiguousarray(W.T)
        wmaps[f"Asd{li}"] = np.concatenate(
            [_blockdiag(np.asarray(inputs[f"att_src{li}"], np.float32)),
             _blockdiag(np.asarray(inputs[f"att_dst{li}"], np.float32))],
            axis=1)
        wmaps[f"Ae{li}"] = _blockdiag(
            np.asarray(inputs[f"att_edge{li}"], np.float32))
        wmaps[f"WeT{li}"] = np.ascontiguousarray(
            np.asarray(inputs[f"W_edge{li}"], np.float32).T)
        wmaps[f"b{li}"] = np.asarray(
            inputs[f"bias{li}"], np.float32).reshape(1, F)

    in_maps = []
    for c in range(NCORES):
        m = dict(wmaps)
        m["xT"] = meta["xts"][c]
        m["aux"] = meta["auxcats"][c].view(np.int16)
        m["dlt"] = meta["dltcats"][c].view(np.int16)
        m["cntinv"] = meta["cntinv"][c]
        m["padc"] = np.full((1, H), PAD_ASRC, BF)
        in_maps.append(m)
    return in_maps


def kernel(x, edge_index, edge_attr,
           W1, att_src1, att_dst1, W_edge1, att_edge1, bias1,
           W2, att_src2, att_dst2, W_edge2, att_edge2, bias2):
    x = np.asarray(x, np.float32)
    edge_attr = np.asarray(edge_attr, np.float32)
    src = np.asarray(edge_index[0], np.int64)
    dst = np.asarray(edge_index[1], np.int64)

    import time
    t0 = time.time()
    meta = _preprocess(x, src, dst, edge_attr)
    t1 = time.time()
    nc = _build(meta)
    t2 = time.time()
    ndesc = sum(n for sec in meta["sections"] for (_, n, _, _, _, _) in sec)
    print(f"preprocess {t1 - t0:.1f}s  build+compile {t2 - t1:.1f}s "
          f"(descs/core/layer {ndesc} = {ndesc * NCORES / meta['E']:.3f}x E/8)",
          flush=True)

    inputs = dict(W1=W1, att_src1=att_src1, att_dst1=att_dst1,
                  W_edge1=W_edge1, att_edge1=att_edge1, bias1=bias1,
                  W2=W2, att_src2=att_src2, att_dst2=att_dst2,
                  W_edge2=W_edge2, att_edge2=att_edge2, bias2=bias2)
    in_maps = _make_in_maps(meta, inputs)

    trace = os.environ.get("GNN_TRACE") == "1"
    t3 = time.time()
    res = run_bass_kernel_spmd(nc, in_maps, list(range(NCORES)), trace=trace)
    print(f"run {time.time() - t3:.1f}s", flush=True)
    if trace and res.exec_time_ns is not None:
        print(f"HW exec time: {res.exec_time_ns} ns", flush=True)

    out = np.zeros((meta["N"], F), dtype=np.float32)
    perm = meta["perm"]
    for c in range(NCORES):
        oc = np.asarray(res.results[c]["out"], np.float32)
        pc = perm[c].reshape(-1)
        mk = pc >= 0
        out[pc[mk]] = oc[mk]
    return out
